# revision 6
# baseline (speedup 1.0000x reference)
"""Trainium2 Bass kernel for nn_CustomFullyConnectedLayer (topk_masking).

Math: reference builds W[r, c] = a[(r-c)%N] * V[(r-c)%N, c]  (P=D=N=4096,
the scatter-add has no collisions), then y = x @ W.T.

So  y[b, r] = sum_c x[b, c] * a[(r-c)%N] * V[(r-c)%N, c].

Sharding (tensor parallel over out_features): core k owns output columns
r in [512k, 512k+512). Define S_k[c, t] = U[(512k+t-c)%N, c] with
U = a[:,None]*V — a wrapped-diagonal band holding exactly 1/8 of V.
Then y[:, 512k:512k+512] = x @ S_k, a plain dense matmul per core.

Host side: Dykstra soft-top-k mask (50 iters over a 4096-vector, trivial),
band gather via a strided view (layout only), fold the (uniform) mask
scalar into x. Device side: tiled 512x4096 @ 4096x512 matmul, float32r.
"""

from contextlib import ExitStack

import numpy as np

import concourse.bacc as bacc
import concourse.bass as bass
import concourse.mybir as mybir
import concourse.tile as tile
from concourse.bass_utils import run_bass_kernel_spmd

N = 4096          # in_features == out_features == P == D
B = 512           # batch
NCORES = 8
TBAND = N // NCORES  # 512 output columns per core
KT = 128          # contraction tile (partition dim)
NKT = N // KT     # 32
MT = 128          # batch tile (psum partition dim)
NMT = B // MT     # 4

TOPK = 3687       # ceil((1-0.1)*4096*4096/4096)
ALPHA_LR = 0.01
NUM_ITER = 50

_NC_CACHE = {}


def _dykstra_mask(alpha: np.ndarray) -> np.ndarray:
    """Faithful float32 replica of reference.sparse_soft_topk_mask_dykstra."""
    s = (alpha.astype(np.float32) / np.float32(ALPHA_LR)).astype(np.float32)
    n = np.float32(s.shape[0])
    k = np.float32(TOPK)
    x = s.copy()
    p = np.zeros_like(s)
    q = np.zeros_like(s)
    for _ in range(NUM_ITER):
        t = x + p
        y = t + (k - np.sum(t, dtype=np.float32)) / n
        p = t - y
        u = y + q
        xn = np.clip(u, np.float32(0.0), np.float32(1.0))
        q = u - xn
        x = xn
    return x


def _build_nc():
    f32 = mybir.dt.float32
    f32r = mybir.dt.float32r

    nc = bacc.Bacc("TRN2", target_bir_lowering=False)
    # xs = [x.T | S_k] concatenated on host: one DMA per k-tile, so each
    # matmul carries at most one sync wait (f32r matmul = fused LDW struct
    # with a tight wait-slot budget in walrus codegen)
    xs = nc.dram_tensor("xs", [N, B + TBAND], f32r, kind="ExternalInput")
    y = nc.dram_tensor("y", [B, TBAND], f32, kind="ExternalOutput")

    with tile.TileContext(nc) as tc, ExitStack() as ctx:
        xpool = ctx.enter_context(tc.tile_pool(name="xp", bufs=4))
        opool = ctx.enter_context(tc.tile_pool(name="op", bufs=2))
        pspool = ctx.enter_context(tc.tile_pool(name="ps", bufs=1, space="PSUM"))

        ps = [pspool.tile([MT, TBAND], f32, tag=f"ps{m}", name=f"ps{m}") for m in range(NMT)]

        for k in range(NKT):
            xst = xpool.tile([KT, B + TBAND], f32r, tag="x", name=f"xst{k}")
            nc.sync.dma_start(out=xst[:], in_=xs[k * KT:(k + 1) * KT, :])
            for m in range(NMT):
                nc.tensor.matmul(
                    ps[m][:],
                    lhsT=xst[:, m * MT:(m + 1) * MT],
                    rhs=xst[:, B:B + TBAND],
                    start=(k == 0),
                    stop=(k == NKT - 1),
                )

        for m in range(NMT):
            ot = opool.tile([MT, TBAND], f32, tag="o", name=f"ot{m}")
            nc.vector.tensor_copy(ot[:], ps[m][:])
            nc.sync.dma_start(out=y[m * MT:(m + 1) * MT, :], in_=ot[:])

    nc.compile()
    return nc


def _get_nc():
    if "nc" not in _NC_CACHE:
        _NC_CACHE["nc"] = _build_nc()
    return _NC_CACHE["nc"]


def _prepare_in_maps(x, V, alpha):
    a = _dykstra_mask(np.asarray(alpha, dtype=np.float32))
    x = np.asarray(x, dtype=np.float32)
    V = np.asarray(V, dtype=np.float32)

    if np.all(a == a[0]):
        # uniform mask (the alpha=const case): fold the scalar into x
        xs = x * np.float32(a[0])
        U = V
    else:
        xs = x
        U = (a[:, None] * V).astype(np.float32)

    xT = np.ascontiguousarray(xs.T)  # [N, B]

    # doubled-rows copy so every wrapped-diagonal band is a plain strided view
    Vd = np.ascontiguousarray(np.concatenate([U, U], axis=0))  # [2N, N]
    flat = Vd.reshape(-1)

    in_maps = []
    for k in range(NCORES):
        base = (N + TBAND * k) * N
        # S_k[c, t] = Vd[N + 512k + t - c, c] = flat[base + c*(1-N) + t*N]
        Sk = np.lib.stride_tricks.as_strided(
            flat[base:], shape=(N, TBAND), strides=((1 - N) * 4, N * 4)
        )
        xs_k = np.concatenate([xT, Sk], axis=1)  # [N, B + TBAND]
        in_maps.append({"xs": np.ascontiguousarray(xs_k)})
    return in_maps


def _run(in_maps, trace=False, **kw):
    nc = _get_nc()
    return run_bass_kernel_spmd(nc, in_maps, list(range(NCORES)), trace=trace, **kw)


def kernel(x, V, alpha):
    in_maps = _prepare_in_maps(x, V, alpha)
    res = _run(in_maps).results
    return np.concatenate([res[k]["y"] for k in range(NCORES)], axis=1)


# revision 7
# speedup vs baseline: 1.4175x; 1.4175x over previous
"""Trainium2 Bass kernel for nn_CustomFullyConnectedLayer (topk_masking).

Math: reference builds W[r, c] = a[(r-c)%N] * V[(r-c)%N, c]  (P=D=N=4096,
the scatter-add has no collisions), then y = x @ W.T.

So  y[b, r] = sum_c x[b, c] * a[(r-c)%N] * V[(r-c)%N, c].

Sharding (tensor parallel over out_features): core k owns output columns
r in [512k, 512k+512). Define S_k[c, t] = U[(512k+t-c)%N, c] with
U = a[:,None]*V — a wrapped-diagonal band holding exactly 1/8 of V.
Then y[:, 512k:512k+512] = x @ S_k, a plain dense matmul per core.

Host side: Dykstra soft-top-k mask (50 iters over a 4096-vector, trivial),
band gather via a strided view (layout only), fold the (uniform) mask
scalar into x. Device side: tiled 512x4096 @ 4096x512 matmul in fp16
(10-bit mantissa, same as tf32) with fp32 PSUM accumulation.
"""

from contextlib import ExitStack

import numpy as np

import concourse.bacc as bacc
import concourse.bass as bass
import concourse.mybir as mybir
import concourse.tile as tile
from concourse.bass_utils import run_bass_kernel_spmd

N = 4096          # in_features == out_features == P == D
B = 512           # batch
NCORES = 8
TBAND = N // NCORES  # 512 output columns per core
KT = 128          # contraction tile (partition dim)
NKT = N // KT     # 32
MT = 128          # batch tile (psum partition dim)
NMT = B // MT     # 4

TOPK = 3687       # ceil((1-0.1)*4096*4096/4096)
ALPHA_LR = 0.01
NUM_ITER = 50

_NC_CACHE = {}


def _dykstra_mask(alpha: np.ndarray) -> np.ndarray:
    """Faithful float32 replica of reference.sparse_soft_topk_mask_dykstra."""
    s = (alpha.astype(np.float32) / np.float32(ALPHA_LR)).astype(np.float32)
    n = np.float32(s.shape[0])
    k = np.float32(TOPK)
    x = s.copy()
    p = np.zeros_like(s)
    q = np.zeros_like(s)
    for _ in range(NUM_ITER):
        t = x + p
        y = t + (k - np.sum(t, dtype=np.float32)) / n
        p = t - y
        u = y + q
        xn = np.clip(u, np.float32(0.0), np.float32(1.0))
        q = u - xn
        x = xn
    return x


def _build_nc():
    f32 = mybir.dt.float32
    f16 = mybir.dt.float16

    nc = bacc.Bacc("TRN2", target_bir_lowering=False)
    # xs = [x.T | S_k] concatenated on host: one DMA per k-tile, so each
    # matmul carries at most one sync wait (f32r matmul = fused LDW struct
    # with a tight wait-slot budget in walrus codegen)
    xs = nc.dram_tensor("xs", [N, B + TBAND], f16, kind="ExternalInput")
    y = nc.dram_tensor("y", [B, TBAND], f32, kind="ExternalOutput")

    with tile.TileContext(nc) as tc, ExitStack() as ctx:
        xpool = ctx.enter_context(tc.tile_pool(name="xp", bufs=6))
        opool = ctx.enter_context(tc.tile_pool(name="op", bufs=2))
        pspool = ctx.enter_context(tc.tile_pool(name="ps", bufs=1, space="PSUM"))

        ps = [pspool.tile([MT, TBAND], f32, tag=f"ps{m}", name=f"ps{m}") for m in range(NMT)]

        for k in range(NKT):
            xst = xpool.tile([KT, B + TBAND], f16, tag="x", name=f"xst{k}")
            nc.sync.dma_start(out=xst[:], in_=xs[k * KT:(k + 1) * KT, :])
            for m in range(NMT):
                nc.tensor.matmul(
                    ps[m][:],
                    lhsT=xst[:, m * MT:(m + 1) * MT],
                    rhs=xst[:, B:B + TBAND],
                    start=(k == 0),
                    stop=(k == NKT - 1),
                )

        for m in range(NMT):
            ot = opool.tile([MT, TBAND], f32, tag="o", name=f"ot{m}")
            nc.vector.tensor_copy(ot[:], ps[m][:])
            nc.sync.dma_start(out=y[m * MT:(m + 1) * MT, :], in_=ot[:])

    nc.compile()
    return nc


def _get_nc():
    if "nc" not in _NC_CACHE:
        _NC_CACHE["nc"] = _build_nc()
    return _NC_CACHE["nc"]


def _prepare_in_maps(x, V, alpha):
    a = _dykstra_mask(np.asarray(alpha, dtype=np.float32))
    x = np.asarray(x, dtype=np.float32)
    V = np.asarray(V, dtype=np.float32)

    if np.all(a == a[0]):
        # uniform mask (the alpha=const case): fold the scalar into x
        xs = x * np.float32(a[0])
        U = V
    else:
        xs = x
        U = (a[:, None] * V).astype(np.float32)

    xT = np.ascontiguousarray(xs.T)  # [N, B]

    # doubled-rows copy so every wrapped-diagonal band is a plain strided view
    Vd = np.ascontiguousarray(np.concatenate([U, U], axis=0))  # [2N, N]
    flat = Vd.reshape(-1)

    in_maps = []
    for k in range(NCORES):
        base = (N + TBAND * k) * N
        # S_k[c, t] = Vd[N + 512k + t - c, c] = flat[base + c*(1-N) + t*N]
        Sk = np.lib.stride_tricks.as_strided(
            flat[base:], shape=(N, TBAND), strides=((1 - N) * 4, N * 4)
        )
        xs_k = np.concatenate([xT, Sk], axis=1).astype(np.float16)  # [N, B+TBAND]
        in_maps.append({"xs": np.ascontiguousarray(xs_k)})
    return in_maps


def _run(in_maps, trace=False, **kw):
    nc = _get_nc()
    return run_bass_kernel_spmd(nc, in_maps, list(range(NCORES)), trace=trace, **kw)


def kernel(x, V, alpha):
    in_maps = _prepare_in_maps(x, V, alpha)
    res = _run(in_maps).results
    return np.concatenate([res[k]["y"] for k in range(NCORES)], axis=1)


# revision 9
# speedup vs baseline: 1.6142x; 1.1388x over previous
"""Trainium2 Bass kernel for nn_CustomFullyConnectedLayer (topk_masking).

Math: reference builds W[r, c] = a[(r-c)%N] * V[(r-c)%N, c]  (P=D=N=4096,
the scatter-add has no collisions), then y = x @ W.T.

So  y[b, r] = sum_c x[b, c] * a[(r-c)%N] * V[(r-c)%N, c].

Sharding (tensor parallel over out_features): core k owns output columns
r in [512k, 512k+512). Define S_k[c, t] = U[(512k+t-c)%N, c] with
U = a[:,None]*V — a wrapped-diagonal band holding exactly 1/8 of V.
Then y[:, 512k:512k+512] = x @ S_k, a plain dense matmul per core.

Host side: Dykstra soft-top-k mask (50 iters over a 4096-vector, trivial),
band gather via a strided view (layout only), fold the (uniform) mask
scalar into x. Device side: tiled 512x4096 @ 4096x512 matmul in fp16
(10-bit mantissa, same as tf32) with fp32 PSUM accumulation.
"""

from contextlib import ExitStack

import numpy as np

import concourse.bacc as bacc
import concourse.bass as bass
import concourse.mybir as mybir
import concourse.tile as tile
from concourse.bass_utils import run_bass_kernel_spmd

N = 4096          # in_features == out_features == P == D
B = 512           # batch
NCORES = 8
TBAND = N // NCORES  # 512 output columns per core
KT = 128          # contraction tile (partition dim)
NKT = N // KT     # 32
MT = 128          # batch tile (psum partition dim)
NMT = B // MT     # 4

TOPK = 3687       # ceil((1-0.1)*4096*4096/4096)
ALPHA_LR = 0.01
NUM_ITER = 50

_NC_CACHE = {}


def _dykstra_mask(alpha: np.ndarray) -> np.ndarray:
    """Faithful float32 replica of reference.sparse_soft_topk_mask_dykstra."""
    s = (alpha.astype(np.float32) / np.float32(ALPHA_LR)).astype(np.float32)
    n = np.float32(s.shape[0])
    k = np.float32(TOPK)
    x = s.copy()
    p = np.zeros_like(s)
    q = np.zeros_like(s)
    for _ in range(NUM_ITER):
        t = x + p
        y = t + (k - np.sum(t, dtype=np.float32)) / n
        p = t - y
        u = y + q
        xn = np.clip(u, np.float32(0.0), np.float32(1.0))
        q = u - xn
        x = xn
    return x


def _build_nc():
    f32 = mybir.dt.float32
    f16 = mybir.dt.float16

    nc = bacc.Bacc("TRN2", target_bir_lowering=False)
    # xs = [x.T | S_k] concatenated on host: one DMA per k-tile, so each
    # matmul carries at most one sync wait (f32r matmul = fused LDW struct
    # with a tight wait-slot budget in walrus codegen)
    xs = nc.dram_tensor("xs", [N, B + TBAND], f16, kind="ExternalInput")
    y = nc.dram_tensor("y", [B, TBAND], f32, kind="ExternalOutput")

    with tile.TileContext(nc) as tc, ExitStack() as ctx:
        xpool = ctx.enter_context(tc.tile_pool(name="xp", bufs=8))
        opool = ctx.enter_context(tc.tile_pool(name="op", bufs=4))
        pspool = ctx.enter_context(tc.tile_pool(name="ps", bufs=1, space="PSUM"))

        ps = [pspool.tile([MT, TBAND], f32, tag=f"ps{m}", name=f"ps{m}") for m in range(NMT)]

        # PE warm-up: ~4us of dummy matmuls during the preamble/first-DMA
        # window so the HAM clock gate reaches 8/8 before the real stream
        # (cold MMs run at 1.2 GHz for the first ~3.4us otherwise).
        wu = xpool.tile([KT, TBAND], f16, tag="wu", name="wu")
        nc.gpsimd.memset(wu[:], 0.0)
        psw = pspool.tile([MT, TBAND], f32, tag="psw", name="psw")
        for w in range(18):
            nc.tensor.matmul(psw[:], lhsT=wu[:, 0:MT], rhs=wu[:],
                             start=True, stop=True)

        for k in range(NKT):
            xst = xpool.tile([KT, B + TBAND], f16, tag="x", name=f"xst{k}")
            nc.sync.dma_start(out=xst[:], in_=xs[k * KT:(k + 1) * KT, :])
            for m in range(NMT):
                nc.tensor.matmul(
                    ps[m][:],
                    lhsT=xst[:, m * MT:(m + 1) * MT],
                    rhs=xst[:, B:B + TBAND],
                    start=(k == 0),
                    stop=(k == NKT - 1),
                )

        for m in range(NMT):
            ot = opool.tile([MT, TBAND], f32, tag="o", name=f"ot{m}")
            nc.vector.tensor_copy(ot[:], ps[m][:])
            nc.sync.dma_start(out=y[m * MT:(m + 1) * MT, :], in_=ot[:])

    nc.compile()
    return nc


def _get_nc():
    if "nc" not in _NC_CACHE:
        _NC_CACHE["nc"] = _build_nc()
    return _NC_CACHE["nc"]


def _prepare_in_maps(x, V, alpha):
    a = _dykstra_mask(np.asarray(alpha, dtype=np.float32))
    x = np.asarray(x, dtype=np.float32)
    V = np.asarray(V, dtype=np.float32)

    if np.all(a == a[0]):
        # uniform mask (the alpha=const case): fold the scalar into x
        xs = x * np.float32(a[0])
        U = V
    else:
        xs = x
        U = (a[:, None] * V).astype(np.float32)

    xT = np.ascontiguousarray(xs.T)  # [N, B]

    # doubled-rows copy so every wrapped-diagonal band is a plain strided view
    Vd = np.ascontiguousarray(np.concatenate([U, U], axis=0))  # [2N, N]
    flat = Vd.reshape(-1)

    in_maps = []
    for k in range(NCORES):
        base = (N + TBAND * k) * N
        # S_k[c, t] = Vd[N + 512k + t - c, c] = flat[base + c*(1-N) + t*N]
        Sk = np.lib.stride_tricks.as_strided(
            flat[base:], shape=(N, TBAND), strides=((1 - N) * 4, N * 4)
        )
        xs_k = np.concatenate([xT, Sk], axis=1).astype(np.float16)  # [N, B+TBAND]
        in_maps.append({"xs": np.ascontiguousarray(xs_k)})
    return in_maps


def _run(in_maps, trace=False, **kw):
    nc = _get_nc()
    return run_bass_kernel_spmd(nc, in_maps, list(range(NCORES)), trace=trace, **kw)


def kernel(x, V, alpha):
    in_maps = _prepare_in_maps(x, V, alpha)
    res = _run(in_maps).results
    return np.concatenate([res[k]["y"] for k in range(NCORES)], axis=1)


# revision 10
# speedup vs baseline: 1.6156x; 1.0009x over previous
"""Trainium2 Bass kernel for nn_CustomFullyConnectedLayer (topk_masking).

Math: reference builds W[r, c] = a[(r-c)%N] * V[(r-c)%N, c]  (P=D=N=4096,
the scatter-add has no collisions), then y = x @ W.T.

So  y[b, r] = sum_c x[b, c] * a[(r-c)%N] * V[(r-c)%N, c].

Sharding (tensor parallel over out_features): core k owns output columns
r in [512k, 512k+512). Define S_k[c, t] = U[(512k+t-c)%N, c] with
U = a[:,None]*V — a wrapped-diagonal band holding exactly 1/8 of V.
Then y[:, 512k:512k+512] = x @ S_k, a plain dense matmul per core.

Host side: Dykstra soft-top-k mask (50 iters over a 4096-vector, trivial),
band gather via a strided view (layout only), fold the (uniform) mask
scalar into x. Device side: tiled 512x4096 @ 4096x512 matmul in fp16
(10-bit mantissa, same as tf32) with fp32 PSUM accumulation.
"""

from contextlib import ExitStack

import numpy as np

import concourse.bacc as bacc
import concourse.bass as bass
import concourse.mybir as mybir
import concourse.tile as tile
from concourse.bass_utils import run_bass_kernel_spmd

N = 4096          # in_features == out_features == P == D
B = 512           # batch
NCORES = 8
TBAND = N // NCORES  # 512 output columns per core
KT = 128          # contraction tile (partition dim)
NKT = N // KT     # 32
MT = 128          # batch tile (psum partition dim)
NMT = B // MT     # 4

TOPK = 3687       # ceil((1-0.1)*4096*4096/4096)
ALPHA_LR = 0.01
NUM_ITER = 50

_NC_CACHE = {}


def _dykstra_mask(alpha: np.ndarray) -> np.ndarray:
    """Faithful float32 replica of reference.sparse_soft_topk_mask_dykstra."""
    s = (alpha.astype(np.float32) / np.float32(ALPHA_LR)).astype(np.float32)
    n = np.float32(s.shape[0])
    k = np.float32(TOPK)
    x = s.copy()
    p = np.zeros_like(s)
    q = np.zeros_like(s)
    for _ in range(NUM_ITER):
        t = x + p
        y = t + (k - np.sum(t, dtype=np.float32)) / n
        p = t - y
        u = y + q
        xn = np.clip(u, np.float32(0.0), np.float32(1.0))
        q = u - xn
        x = xn
    return x


def _build_nc():
    f32 = mybir.dt.float32
    f16 = mybir.dt.float16

    nc = bacc.Bacc("TRN2", target_bir_lowering=False)
    # xs = [x.T | S_k] concatenated on host: one DMA per k-tile, so each
    # matmul carries at most one sync wait (f32r matmul = fused LDW struct
    # with a tight wait-slot budget in walrus codegen)
    xs = nc.dram_tensor("xs", [N, B + TBAND], f16, kind="ExternalInput")
    y = nc.dram_tensor("y", [B, TBAND], f32, kind="ExternalOutput")

    with tile.TileContext(nc) as tc, ExitStack() as ctx:
        xpool = ctx.enter_context(tc.tile_pool(name="xp", bufs=8))
        opool = ctx.enter_context(tc.tile_pool(name="op", bufs=4))
        pspool = ctx.enter_context(tc.tile_pool(name="ps", bufs=1, space="PSUM"))

        ps = [pspool.tile([MT, TBAND], f32, tag=f"ps{m}", name=f"ps{m}") for m in range(NMT)]

        # PE warm-up: ~4us of dummy matmuls during the preamble/first-DMA
        # window so the HAM clock gate reaches 8/8 before the real stream
        # (cold MMs run at 1.2 GHz for the first ~3.4us otherwise).
        wu = xpool.tile([KT, TBAND], f16, tag="wu", name="wu")
        nc.gpsimd.memset(wu[:], 0.0)
        psw = pspool.tile([MT, TBAND], f32, tag="psw", name="psw")
        for w in range(7):
            nc.tensor.matmul(psw[:], lhsT=wu[:, 0:MT], rhs=wu[:],
                             start=True, stop=True)

        for k in range(NKT):
            xst = xpool.tile([KT, B + TBAND], f16, tag="x", name=f"xst{k}")
            nc.sync.dma_start(out=xst[:], in_=xs[k * KT:(k + 1) * KT, :])
            for m in range(NMT):
                nc.tensor.matmul(
                    ps[m][:],
                    lhsT=xst[:, m * MT:(m + 1) * MT],
                    rhs=xst[:, B:B + TBAND],
                    start=(k == 0),
                    stop=(k == NKT - 1),
                )

        for m in range(NMT):
            ot = opool.tile([MT, TBAND], f32, tag="o", name=f"ot{m}")
            nc.vector.tensor_copy(ot[:], ps[m][:])
            nc.sync.dma_start(out=y[m * MT:(m + 1) * MT, :], in_=ot[:])

    nc.compile()
    return nc


def _get_nc():
    if "nc" not in _NC_CACHE:
        _NC_CACHE["nc"] = _build_nc()
    return _NC_CACHE["nc"]


def _prepare_in_maps(x, V, alpha):
    a = _dykstra_mask(np.asarray(alpha, dtype=np.float32))
    x = np.asarray(x, dtype=np.float32)
    V = np.asarray(V, dtype=np.float32)

    if np.all(a == a[0]):
        # uniform mask (the alpha=const case): fold the scalar into x
        xs = x * np.float32(a[0])
        U = V
    else:
        xs = x
        U = (a[:, None] * V).astype(np.float32)

    xT = np.ascontiguousarray(xs.T)  # [N, B]

    # doubled-rows copy so every wrapped-diagonal band is a plain strided view
    Vd = np.ascontiguousarray(np.concatenate([U, U], axis=0))  # [2N, N]
    flat = Vd.reshape(-1)

    in_maps = []
    for k in range(NCORES):
        base = (N + TBAND * k) * N
        # S_k[c, t] = Vd[N + 512k + t - c, c] = flat[base + c*(1-N) + t*N]
        Sk = np.lib.stride_tricks.as_strided(
            flat[base:], shape=(N, TBAND), strides=((1 - N) * 4, N * 4)
        )
        xs_k = np.concatenate([xT, Sk], axis=1).astype(np.float16)  # [N, B+TBAND]
        in_maps.append({"xs": np.ascontiguousarray(xs_k)})
    return in_maps


def _run(in_maps, trace=False, **kw):
    nc = _get_nc()
    return run_bass_kernel_spmd(nc, in_maps, list(range(NCORES)), trace=trace, **kw)


def kernel(x, V, alpha):
    in_maps = _prepare_in_maps(x, V, alpha)
    res = _run(in_maps).results
    return np.concatenate([res[k]["y"] for k in range(NCORES)], axis=1)
